# revision 1
# baseline (speedup 1.0000x reference)
"""BitNet dense layer on 8 Trainium2 NeuronCores.

reference math:
    row_scale = clip(mean(|W|, axis=1), 1e-8)        # [out]
    out = (x @ sign(W).T) * row_scale * scale_param  # [B,S,out]

Strategy (data-parallel over the 8192 tokens):
  * Host folds row_scale * scale_param into the binarized weight:
        Wf = sign(W) * comb[:, None]   -> bf16, exactly +-comb[o] per row
    so the device kernel is a single plain matmul.
  * Host pre-transposes both operands so the device streams natural-layout
    [K, *] tiles (contraction dim on partitions) with zero on-chip transposes:
        xT [4096, 8192] bf16 (sharded 1024 tokens/core), wT [4096, 4096] bf16.
  * Each core computes out_c[1024, 4096] f32 = xT_c.T @ wT via the production
    tile matmul kernel; host concatenates the 8 shards.
"""

import numpy as np
import ml_dtypes

B, S, D_IN, D_OUT = 4, 2048, 4096, 4096
N_CORES = 8
M_TOT = B * S
M_LOC = M_TOT // N_CORES

_prog = None
last_results = None  # BassKernelResults of the most recent run (for test harness)
TRACE = False  # set True by the dev test harness (needs NTFF shims) to profile


def _build_program():
    import concourse.tile as tile
    from concourse import bacc, mybir
    from concourse.kernels.tile_matmul import matmul_tile_kernel

    nc = bacc.Bacc(
        "TRN2", target_bir_lowering=False, debug=False, num_devices=N_CORES
    )
    xT = nc.dram_tensor(
        "xT", [D_IN, M_LOC], mybir.dt.bfloat16, kind="ExternalInput"
    ).ap()
    wT = nc.dram_tensor(
        "wT", [D_IN, D_OUT], mybir.dt.bfloat16, kind="ExternalInput"
    ).ap()
    out = nc.dram_tensor(
        "out", [M_LOC, D_OUT], mybir.dt.float32, kind="ExternalOutput"
    ).ap()
    with tile.TileContext(nc) as tc:
        # PE warmup: dummy matmuls run while the first real tiles DMA in,
        # releasing the HAM clock gate (1.2 -> 2.4 GHz takes ~3.4us of PE
        # activity) so the real matmul stream starts at full clock. Sized to
        # END before the first real tiles land (~14us): PE executes in order,
        # so a longer warmup would gate the real stream on itself. Memsets go
        # to DVE explicitly so the warmup starts right after engine preamble.
        with (
            tc.tile_pool(name="warm", bufs=1) as warm,
            tc.tile_pool(name="warm_psum", bufs=1, space="PSUM") as warm_psum,
        ):
            wa = warm.tile([128, 128], mybir.dt.bfloat16)
            wb = warm.tile([128, 512], mybir.dt.bfloat16)
            nc.vector.memset(wa[:], 0.0)
            nc.vector.memset(wb[:], 0.0)
            ps = warm_psum.tile([128, 512], mybir.dt.float32)
            for i in range(10):
                nc.tensor.matmul(ps[:], wa[:], wb[:], start=(i == 0), stop=(i == 9))
        matmul_tile_kernel(
            tc,
            kxm_ap=xT,
            kxn_ap=wT,
            mxn_ap=out,
            # PSUM evictions on the (otherwise idle) DVE: faster than the ACT
            # copy default, shortening the end-of-kernel eviction->DMA chain.
            psum_evict_fn=lambda nc_, psum, sbuf: nc_.vector.tensor_copy(
                out=sbuf, in_=psum
            ),
        )
    nc.compile()
    return nc


def kernel(input, weight, scale_param):
    global _prog, last_results
    from concourse.bass_utils import run_bass_kernel_spmd

    x = np.asarray(input, dtype=np.float32).reshape(M_TOT, D_IN)
    W = np.asarray(weight, dtype=np.float32)
    sp = np.asarray(scale_param, dtype=np.float32)

    comb = np.clip(np.abs(W).mean(axis=1, dtype=np.float32), 1e-8, None) * sp
    wT = (np.sign(W) * comb[:, None].astype(np.float32)).T.astype(
        ml_dtypes.bfloat16, order="C"
    )
    xT = x.T.astype(ml_dtypes.bfloat16, order="C")

    if _prog is None:
        _prog = _build_program()

    in_maps = [
        {
            "xT": np.ascontiguousarray(xT[:, c * M_LOC : (c + 1) * M_LOC]),
            "wT": wT,
        }
        for c in range(N_CORES)
    ]
    last_results = run_bass_kernel_spmd(
        _prog, in_maps, list(range(N_CORES)), trace=TRACE
    )
    out = np.concatenate(
        [last_results.results[c]["out"] for c in range(N_CORES)], axis=0
    )
    return np.nan_to_num(
        out.reshape(B, S, D_OUT), nan=0.0, posinf=1e6, neginf=-1e6
    )



# revision 2
# speedup vs baseline: 1.2021x; 1.2021x over previous
"""BitNet dense layer on 8 Trainium2 NeuronCores.

reference math:
    row_scale = clip(mean(|W|, axis=1), 1e-8)        # [out]
    out = (x @ sign(W).T) * row_scale * scale_param  # [B,S,out]

Strategy (data-parallel over the 8192 tokens, split-K mixed precision):
  * The binary weight is exactly representable in fp8 (+-1), and the 2e-2
    error budget is ~17x the bf16 activation error, so half the contraction
    dim runs through the fp8 DoubleRow path (157 TF/s, 2x bf16):
        out = x8[:, :K8] @ S8 + xb[:, K8:] @ Sb      (sign domain, fp32 psum)
    with x8 = e4m3(x) (rel err 2^-4 -> max_rel 0.021*sqrt(K8/K) ~= 0.014),
    xb = bf16(x). Host applies the exact fp32 row scale afterwards:
        out *= row_scale * scale_param
  * Host pre-transposes operands so the device streams natural-layout
    [K, *] tiles (contraction on partitions) with zero on-chip transposes.
  * Each core computes out8+outb for its 1024 tokens via two back-to-back
    tile matmuls (fp8 then bf16) in one program; host sums + scales.
"""

import numpy as np
import ml_dtypes

B, S, D_IN, D_OUT = 4, 2048, 4096, 4096
N_CORES = 8
M_TOT = B * S
M_LOC = M_TOT // N_CORES
K8 = 2048  # contraction columns routed through fp8 DoubleRow
KB = D_IN - K8

_prog = None
last_results = None  # BassKernelResults of the most recent run (for test harness)
TRACE = False  # set True by the dev test harness (needs NTFF shims) to profile


def _build_program():
    import concourse.tile as tile
    from concourse import bacc, mybir
    from concourse.kernels.tile_matmul import matmul_tile_kernel

    nc = bacc.Bacc(
        "TRN2", target_bir_lowering=False, debug=False, num_devices=N_CORES
    )
    xT8 = nc.dram_tensor(
        "xT8", [K8, M_LOC], mybir.dt.float8e4, kind="ExternalInput"
    ).ap()
    wT8 = nc.dram_tensor(
        "wT8", [K8, D_OUT], mybir.dt.float8e4, kind="ExternalInput"
    ).ap()
    xTb = nc.dram_tensor(
        "xTb", [KB, M_LOC], mybir.dt.bfloat16, kind="ExternalInput"
    ).ap()
    wTb = nc.dram_tensor(
        "wTb", [KB, D_OUT], mybir.dt.bfloat16, kind="ExternalInput"
    ).ap()
    out8 = nc.dram_tensor(
        "out8", [M_LOC, D_OUT], mybir.dt.float32, kind="ExternalOutput"
    ).ap()
    outb = nc.dram_tensor(
        "outb", [M_LOC, D_OUT], mybir.dt.float32, kind="ExternalOutput"
    ).ap()
    with tile.TileContext(nc) as tc:
        # PE warmup: dummy matmuls run while the first real tiles DMA in,
        # releasing the HAM clock gate (1.2 -> 2.4 GHz takes ~3.4us of PE
        # activity) so the real matmul stream starts at full clock. Sized to
        # END before the first real tiles land: PE executes in order, so a
        # longer warmup would gate the real stream on itself. Memsets go to
        # DVE explicitly so the warmup starts right after engine preamble.
        with (
            tc.tile_pool(name="warm", bufs=1) as warm,
            tc.tile_pool(name="warm_psum", bufs=1, space="PSUM") as warm_psum,
        ):
            wa = warm.tile([128, 128], mybir.dt.bfloat16)
            wb = warm.tile([128, 512], mybir.dt.bfloat16)
            nc.vector.memset(wa[:], 0.0)
            nc.vector.memset(wb[:], 0.0)
            ps = warm_psum.tile([128, 512], mybir.dt.float32)
            for i in range(10):
                nc.tensor.matmul(ps[:], wa[:], wb[:], start=(i == 0), stop=(i == 9))
        # PSUM evictions on the (otherwise idle) DVE: faster than the ACT
        # copy default, shortening the end-of-kernel eviction->DMA chain.
        evict = lambda nc_, psum, sbuf: nc_.vector.tensor_copy(out=sbuf, in_=psum)
        matmul_tile_kernel(
            tc,
            kxm_ap=xT8,
            kxn_ap=wT8,
            mxn_ap=out8,
            psum_evict_fn=evict,
        )
        matmul_tile_kernel(
            tc,
            kxm_ap=xTb,
            kxn_ap=wTb,
            mxn_ap=outb,
            psum_evict_fn=evict,
        )
    nc.compile()
    return nc


def kernel(input, weight, scale_param):
    global _prog, last_results
    from concourse.bass_utils import run_bass_kernel_spmd

    x = np.asarray(input, dtype=np.float32).reshape(M_TOT, D_IN)
    W = np.asarray(weight, dtype=np.float32)
    sp = np.asarray(scale_param, dtype=np.float32)

    comb = np.clip(np.abs(W).mean(axis=1, dtype=np.float32), 1e-8, None) * sp
    ST = np.sign(W).T  # [in, out], exact +-1/0
    wT8 = ST[:K8].astype(ml_dtypes.float8_e4m3, order="C")
    wTb = ST[K8:].astype(ml_dtypes.bfloat16, order="C")
    xT = x.T
    xT8 = xT[:K8].astype(ml_dtypes.float8_e4m3, order="C")
    xTb = xT[K8:].astype(ml_dtypes.bfloat16, order="C")

    if _prog is None:
        _prog = _build_program()

    in_maps = [
        {
            "xT8": np.ascontiguousarray(xT8[:, c * M_LOC : (c + 1) * M_LOC]),
            "wT8": wT8,
            "xTb": np.ascontiguousarray(xTb[:, c * M_LOC : (c + 1) * M_LOC]),
            "wTb": wTb,
        }
        for c in range(N_CORES)
    ]
    last_results = run_bass_kernel_spmd(
        _prog, in_maps, list(range(N_CORES)), trace=TRACE
    )
    out = np.concatenate(
        [
            last_results.results[c]["out8"] + last_results.results[c]["outb"]
            for c in range(N_CORES)
        ],
        axis=0,
    )
    out *= comb[None, :]
    return np.nan_to_num(
        out.reshape(B, S, D_OUT), nan=0.0, posinf=1e6, neginf=-1e6
    )


# revision 3
# speedup vs baseline: 1.3116x; 1.0911x over previous
"""BitNet dense layer on 8 Trainium2 NeuronCores.

reference math:
    row_scale = clip(mean(|W|, axis=1), 1e-8)        # [out]
    out = (x @ sign(W).T) * row_scale * scale_param  # [B,S,out]

Strategy (data-parallel over the 8192 tokens, split-K mixed precision):
  * The binary weight is exactly representable in fp8 (+-1), and the 2e-2
    error budget is ~17x the bf16 activation error, so half the contraction
    dim runs through the fp8 DoubleRow path (157 TF/s, 2x bf16):
        out = x8[:, :K8] @ S8 + xb[:, K8:] @ Sb      (sign domain, fp32 psum)
    with x8 = e4m3(x) (rel err 2^-4 -> max_rel 0.021*sqrt(K8/K) ~= 0.014),
    xb = bf16(x). Host applies the exact fp32 row scale afterwards:
        out *= row_scale * scale_param
  * Both halves run as ONE composable tile matmul with two K-batches, so
    fp8 and bf16 partials accumulate into the same PSUM group: single
    output tensor, no inter-kernel bubble, one eviction tail.
  * PSUM double-buffered (2 x 4 banks) so evictions overlap the next
    n-tile's accumulation.
  * Host pre-transposes operands so the device streams natural-layout
    [K, *] tiles (contraction on partitions) with zero on-chip transposes.
"""

import numpy as np
import ml_dtypes

B, S, D_IN, D_OUT = 4, 2048, 4096, 4096
N_CORES = 8
M_TOT = B * S
M_LOC = M_TOT // N_CORES
K8 = 2048  # contraction columns routed through fp8 DoubleRow
KB = D_IN - K8

_prog = None
last_results = None  # BassKernelResults of the most recent run (for test harness)
TRACE = False  # set True by the dev test harness (needs NTFF shims) to profile


def _build_program():
    import concourse.tile as tile
    from concourse import bacc, mybir
    from concourse.kernels.tile_matmul import (
        batched_producer_kxm,
        batched_producer_kxn,
        composable_matmul_tile_kernel,
        dma_from_dram_kxm,
        dma_from_dram_kxn,
        dma_to_dram_mxn,
    )

    nc = bacc.Bacc(
        "TRN2", target_bir_lowering=False, debug=False, num_devices=N_CORES
    )
    xT8 = nc.dram_tensor(
        "xT8", [K8, M_LOC], mybir.dt.float8e4, kind="ExternalInput"
    ).ap()
    wT8 = nc.dram_tensor(
        "wT8", [K8, D_OUT], mybir.dt.float8e4, kind="ExternalInput"
    ).ap()
    xTb = nc.dram_tensor(
        "xTb", [KB, M_LOC], mybir.dt.bfloat16, kind="ExternalInput"
    ).ap()
    wTb = nc.dram_tensor(
        "wTb", [KB, D_OUT], mybir.dt.bfloat16, kind="ExternalInput"
    ).ap()
    out = nc.dram_tensor(
        "out", [M_LOC, D_OUT], mybir.dt.float32, kind="ExternalOutput"
    ).ap()
    with tile.TileContext(nc) as tc:
        # PE warmup: dummy matmuls run while the first real tiles DMA in,
        # releasing the HAM clock gate (1.2 -> 2.4 GHz takes ~3.4us of PE
        # activity) so the real matmul stream starts at full clock. Sized to
        # END before the first real tiles land: PE executes in order, so a
        # longer warmup would gate the real stream on itself. Memsets go to
        # DVE explicitly so the warmup starts right after engine preamble.
        with (
            tc.tile_pool(name="warm", bufs=1) as warm,
            tc.tile_pool(name="warm_psum", bufs=1, space="PSUM") as warm_psum,
        ):
            wa = warm.tile([128, 128], mybir.dt.bfloat16)
            wb = warm.tile([128, 512], mybir.dt.bfloat16)
            nc.vector.memset(wa[:], 0.0)
            nc.vector.memset(wb[:], 0.0)
            ps = warm_psum.tile([128, 512], mybir.dt.float32)
            for i in range(10):
                nc.tensor.matmul(ps[:], wa[:], wb[:], start=(i == 0), stop=(i == 9))

        tc.swap_default_side()
        with (
            tc.tile_pool(name="kxm8", bufs=5) as kxm8_pool,
            tc.tile_pool(name="kxmb", bufs=5) as kxmb_pool,
            tc.tile_pool(name="kxn8", bufs=5) as kxn8_pool,
            tc.tile_pool(name="kxnb", bufs=5) as kxnb_pool,
        ):
            p8m, s8m = dma_from_dram_kxm(kxm8_pool, xT8)
            pbm, sbm = dma_from_dram_kxm(kxmb_pool, xTb)
            kxm_producer, kxm_shape = batched_producer_kxm(
                [p8m, pbm], [s8m, sbm], batch_dim="k"
            )
            p8n, s8n = dma_from_dram_kxn(kxn8_pool, wT8)
            pbn, sbn = dma_from_dram_kxn(kxnb_pool, wTb)
            kxn_producer, kxn_shape = batched_producer_kxn(
                [p8n, pbn], [s8n, sbn], batch_dim="k"
            )
            composable_matmul_tile_kernel(
                tc=tc,
                kxm_shape=kxm_shape,
                kxn_shape=kxn_shape,
                output_type=mybir.dt.float32,
                kxm_producer=kxm_producer,
                kxn_producer=kxn_producer,
                mxn_consumer=dma_to_dram_mxn(out),
                # PSUM evictions on the (otherwise idle) DVE: faster than the
                # ACT copy default, shortening the eviction->DMA chain.
                mxn_subtile_reducer=lambda nc_, psum, sbuf, md: (
                    nc_.vector.tensor_copy(out=sbuf, in_=psum)
                ),
                psum_n_bufs=2,
            )
    nc.compile()
    return nc


def kernel(input, weight, scale_param):
    global _prog, last_results
    from concourse.bass_utils import run_bass_kernel_spmd

    x = np.asarray(input, dtype=np.float32).reshape(M_TOT, D_IN)
    W = np.asarray(weight, dtype=np.float32)
    sp = np.asarray(scale_param, dtype=np.float32)

    comb = np.clip(np.abs(W).mean(axis=1, dtype=np.float32), 1e-8, None) * sp
    ST = np.sign(W).T  # [in, out], exact +-1/0
    wT8 = ST[:K8].astype(ml_dtypes.float8_e4m3, order="C")
    wTb = ST[K8:].astype(ml_dtypes.bfloat16, order="C")
    xT = x.T
    xT8 = xT[:K8].astype(ml_dtypes.float8_e4m3, order="C")
    xTb = xT[K8:].astype(ml_dtypes.bfloat16, order="C")

    if _prog is None:
        _prog = _build_program()

    in_maps = [
        {
            "xT8": np.ascontiguousarray(xT8[:, c * M_LOC : (c + 1) * M_LOC]),
            "wT8": wT8,
            "xTb": np.ascontiguousarray(xTb[:, c * M_LOC : (c + 1) * M_LOC]),
            "wTb": wTb,
        }
        for c in range(N_CORES)
    ]
    last_results = run_bass_kernel_spmd(
        _prog, in_maps, list(range(N_CORES)), trace=TRACE
    )
    out = np.concatenate(
        [last_results.results[c]["out"] for c in range(N_CORES)], axis=0
    )
    out *= comb[None, :]
    return np.nan_to_num(
        out.reshape(B, S, D_OUT), nan=0.0, posinf=1e6, neginf=-1e6
    )


# revision 7
# speedup vs baseline: 1.3248x; 1.0100x over previous
"""BitNet dense layer on 8 Trainium2 NeuronCores.

reference math:
    row_scale = clip(mean(|W|, axis=1), 1e-8)        # [out]
    out = (x @ sign(W).T) * row_scale * scale_param  # [B,S,out]

Strategy (data-parallel over the 8192 tokens, split-K mixed precision):
  * The binary weight is exactly representable in fp8 (+-1), and the 2e-2
    error budget is ~17x the bf16 activation error, so half the contraction
    dim runs through the fp8 DoubleRow path (157 TF/s, 2x bf16):
        out = x8[:, :K8] @ S8 + xb[:, K8:] @ Sb      (sign domain, fp32 psum)
    with x8 = e4m3(x) (rel err 2^-4 -> max_rel 0.021*sqrt(K8/K) ~= 0.014),
    xb = bf16(x). Host applies the exact fp32 row scale afterwards:
        out *= row_scale * scale_param
  * Both halves run as ONE composable tile matmul with two K-batches, so
    fp8 and bf16 partials accumulate into the same PSUM group: single
    output tensor, no inter-kernel bubble, one eviction tail.
  * PSUM double-buffered (2 x 4 banks) so evictions overlap the next
    n-tile's accumulation.
  * Host pre-transposes operands so the device streams natural-layout
    [K, *] tiles (contraction on partitions) with zero on-chip transposes.
"""

import numpy as np
import ml_dtypes

B, S, D_IN, D_OUT = 4, 2048, 4096, 4096
N_CORES = 8
M_TOT = B * S
M_LOC = M_TOT // N_CORES
K8 = 2048  # contraction columns routed through fp8 DoubleRow
KB = D_IN - K8

_prog = None
last_results = None  # BassKernelResults of the most recent run (for test harness)
TRACE = False  # set True by the dev test harness (needs NTFF shims) to profile


def _build_program():
    import concourse.tile as tile
    from concourse import bacc, mybir
    from concourse.kernels.tile_matmul import (
        batched_producer_kxm,
        batched_producer_kxn,
        composable_matmul_tile_kernel,
        dma_from_dram_kxm,
        dma_from_dram_kxn,
    )

    nc = bacc.Bacc(
        "TRN2", target_bir_lowering=False, debug=False, num_devices=N_CORES
    )
    xT8 = nc.dram_tensor(
        "xT8", [K8, M_LOC], mybir.dt.float8e4, kind="ExternalInput"
    ).ap()
    wT8 = nc.dram_tensor(
        "wT8", [K8, D_OUT], mybir.dt.float8e4, kind="ExternalInput"
    ).ap()
    xTb = nc.dram_tensor(
        "xTb", [KB, M_LOC], mybir.dt.bfloat16, kind="ExternalInput"
    ).ap()
    wTb = nc.dram_tensor(
        "wTb", [KB, D_OUT], mybir.dt.bfloat16, kind="ExternalInput"
    ).ap()
    out = nc.dram_tensor(
        "out", [M_LOC, D_OUT], mybir.dt.float32, kind="ExternalOutput"
    ).ap()
    with tile.TileContext(nc) as tc:
        # PE warmup: dummy matmuls run while the first real tiles DMA in,
        # releasing the HAM clock gate (1.2 -> 2.4 GHz takes ~3.4us of PE
        # activity) so the real matmul stream starts at full clock. Sized to
        # END before the first real tiles land: PE executes in order, so a
        # longer warmup would gate the real stream on itself. Memsets go to
        # DVE explicitly so the warmup starts right after engine preamble.
        with (
            tc.tile_pool(name="warm", bufs=1) as warm,
            tc.tile_pool(name="warm_psum", bufs=1, space="PSUM") as warm_psum,
        ):
            wa = warm.tile([128, 128], mybir.dt.bfloat16)
            wb = warm.tile([128, 512], mybir.dt.bfloat16)
            # Memsets on GPSIMD: it comes out of the engine preamble ~1.5us
            # before DVE's first slot, so the warmup matmuls (which wait on
            # these) issue that much earlier.
            nc.gpsimd.memset(wa[:], 0.0)
            nc.gpsimd.memset(wb[:], 0.0)
            ps = warm_psum.tile([128, 512], mybir.dt.float32)
            for i in range(10):
                nc.tensor.matmul(ps[:], wa[:], wb[:], start=(i == 0), stop=(i == 9))

        tc.swap_default_side()
        with (
            tc.tile_pool(name="kxm8", bufs=5) as kxm8_pool,
            tc.tile_pool(name="kxmb", bufs=5) as kxmb_pool,
            tc.tile_pool(name="kxn8", bufs=5) as kxn8_pool,
            tc.tile_pool(name="kxnb", bufs=5) as kxnb_pool,
        ):
            p8m, s8m = dma_from_dram_kxm(kxm8_pool, xT8)
            pbm, sbm = dma_from_dram_kxm(kxmb_pool, xTb)
            kxm_producer, kxm_shape = batched_producer_kxm(
                [p8m, pbm], [s8m, sbm], batch_dim="k"
            )
            p8n, s8n = dma_from_dram_kxn(kxn8_pool, wT8)
            pbn, sbn = dma_from_dram_kxn(kxnb_pool, wTb)
            kxn_producer, kxn_shape = batched_producer_kxn(
                [p8n, pbn], [s8n, sbn], batch_dim="k"
            )

            from concourse.bass import ds, ts

            out3d = out.rearrange("(po pi) f -> pi po f", pi=128)

            def consumer(nc_, mxn_tile, md):
                # One DMA per m-subtile instead of one per tile: each write
                # depends only on its own subtile's eviction, so the final
                # evict->DMA chain pipelines instead of serializing.
                for i in range(mxn_tile.shape[1]):
                    nc_.sync.dma_start(
                        out3d[
                            :,
                            md.m_tile_idx * md.m_subtiles + i,
                            ds(md.n_tile_idx * md.n_tile, md.n_tile),
                        ],
                        mxn_tile[:, i, : md.n_tile],
                    )

            def reducer(nc_, psum, sbuf, md):
                # PSUM evictions alternate between DVE and ACT (GPSIMD cannot
                # read PSUM) so consecutive evictions run in parallel.
                if md.m_subtile_idx % 2 == 0:
                    nc_.vector.tensor_copy(out=sbuf, in_=psum)
                else:
                    nc_.scalar.activation(
                        sbuf, psum, mybir.ActivationFunctionType.Copy
                    )

            composable_matmul_tile_kernel(
                tc=tc,
                kxm_shape=kxm_shape,
                kxn_shape=kxn_shape,
                output_type=mybir.dt.float32,
                kxm_producer=kxm_producer,
                kxn_producer=kxn_producer,
                mxn_consumer=consumer,
                mxn_subtile_reducer=reducer,
                psum_n_bufs=2,
            )
    nc.compile()
    return nc


def kernel(input, weight, scale_param):
    global _prog, last_results
    from concourse.bass_utils import run_bass_kernel_spmd

    x = np.asarray(input, dtype=np.float32).reshape(M_TOT, D_IN)
    W = np.asarray(weight, dtype=np.float32)
    sp = np.asarray(scale_param, dtype=np.float32)

    comb = np.clip(np.abs(W).mean(axis=1, dtype=np.float32), 1e-8, None) * sp
    ST = np.sign(W).T  # [in, out], exact +-1/0
    wT8 = ST[:K8].astype(ml_dtypes.float8_e4m3, order="C")
    wTb = ST[K8:].astype(ml_dtypes.bfloat16, order="C")
    xT = x.T
    xT8 = xT[:K8].astype(ml_dtypes.float8_e4m3, order="C")
    xTb = xT[K8:].astype(ml_dtypes.bfloat16, order="C")

    if _prog is None:
        _prog = _build_program()

    in_maps = [
        {
            "xT8": np.ascontiguousarray(xT8[:, c * M_LOC : (c + 1) * M_LOC]),
            "wT8": wT8,
            "xTb": np.ascontiguousarray(xTb[:, c * M_LOC : (c + 1) * M_LOC]),
            "wTb": wTb,
        }
        for c in range(N_CORES)
    ]
    last_results = run_bass_kernel_spmd(
        _prog, in_maps, list(range(N_CORES)), trace=TRACE
    )
    out = np.concatenate(
        [last_results.results[c]["out"] for c in range(N_CORES)], axis=0
    )
    out *= comb[None, :]
    return np.nan_to_num(
        out.reshape(B, S, D_OUT), nan=0.0, posinf=1e6, neginf=-1e6
    )
